# revision 3
# baseline (speedup 1.0000x reference)
"""STConvBlock Trainium2 kernel v4.

vs v2: resident fp8 uneg masks (no per-(j,k) streaming), al staged with one
DMA + partition broadcasts, ELU relu on DVE, and a SHARDED TAIL: core c
computes only output pairs {2c, 2c+1} via a zero-padded 20-tap GLU2 conv
(per-core weights w2sel bake the 3 taps + residual at the right slice
offsets), writing a per-core [2, C, N] output that the host assembles.

Sharding: 40 (slice, head) units; core c owns units [4c..4c+3, 32+c].
AllGather per local unit j into shared agos[j].

Score math per (slice xs, head, cheb k), tile [128 j, 1024 i] (S transposed):
  v_r = al_r[i] + ar_r[j]    (DVE tensor_scalar: al broadcast + per-part ar)
  t_r = v_r * m_r            (DVE tensor_tensor, masks resident bf16)
  ss  = t0+t1+t2             (PE: 3 identity-matmul injects into f32 PSUM)
  X   = exp(ss)              (ACT, PSUM -> SBUF bf16; off-union X == 1.0)
  num/den += wxo^T @ X + wxo^T @ (u-1)   (PE; exact cancel off-union)
"""

import os
import numpy as np
import ml_dtypes

B, T, N, C = 2, 12, 1024, 64
KT = 3
T1 = T - KT + 1   # 10
T2 = T1 - KT + 1  # 8
H, K1, R = 2, 3, 2
NSLICE = B * T1       # 20
NUNITS = NSLICE * H   # 40
N_CORES = 8
NT = N // 128         # 8
FCH = 512
NF = N // FCH         # 2
NC_ELEMS = float(N * C)
UPC = NUNITS // N_CORES  # 5
NSL = 3                  # distinct slices per core
PAIRS_PC = 2             # tail (b,t2) pairs per core
J2S = [2, 0, 0, 1, 1]

_cache = {}
POOL_R2 = int(os.environ.get('K4_POOL_R2', '1'))


def _build(n_cores, triv_gb=True, debug=False, reps=1):
    import concourse.bass as bass
    import concourse.tile as tile
    import concourse.mybir as mybir
    from concourse import bacc
    from concourse.masks import make_identity

    F32 = mybir.dt.float32
    BF16 = mybir.dt.bfloat16
    FP8 = mybir.dt.float8e4
    AF = mybir.ActivationFunctionType
    ALU = mybir.AluOpType
    AX = mybir.AxisListType

    nc = bacc.Bacc(None, target_bir_lowering=False)
    xw = nc.dram_tensor("xw", [NSL, C, KT, N], BF16, kind="ExternalInput")
    w1T = nc.dram_tensor("w1T", [KT, C, 2 * C], BF16, kind="ExternalInput")
    res1 = nc.dram_tensor("res1", [C, 2 * C], BF16, kind="ExternalInput")
    wlr = nc.dram_tensor("wlr", [UPC, K1, C, K1], BF16, kind="ExternalInput")
    wrt = nc.dram_tensor("wrt", [UPC, K1, C, K1 + C], BF16, kind="ExternalInput")
    mrelT = nc.dram_tensor("mrelT", [R, N, N], BF16, kind="ExternalInput")
    msupT = nc.dram_tensor("msupT", [K1, N, N], BF16, kind="ExternalInput")
    unegT = nc.dram_tensor("unegT", [K1, N, N], FP8, kind="ExternalInput")
    # per-core zero-padded GLU2 weights: taps + residual at slice offsets
    w2sel = nc.dram_tensor("w2sel", [PAIRS_PC, NSLICE, C, 2 * C], BF16,
                           kind="ExternalInput")
    gbT = nc.dram_tensor("gbT", [PAIRS_PC, 2, C, N], BF16, kind="ExternalInput")
    out = nc.dram_tensor("out", [PAIRS_PC, C, N], F32, kind="ExternalOutput")
    ag_in = nc.dram_tensor("ag_in", [3, C, N], BF16)
    # agos_av[q][c] = av slice 2c+q (head-summed, pre-averaged by w2sel);
    # agos_ex[c] = unit 32+c (slices 16..19, heads split across cores)
    agos_av = [nc.dram_tensor(f"agoav{q}", [N_CORES, C, N], BF16,
                              addr_space="Shared") for q in range(2)]
    agos_ex = nc.dram_tensor("agoex", [N_CORES, C, N], BF16,
                             addr_space="Shared")

    with tile.TileContext(nc) as tc:
        with (
            tc.tile_pool(name="consts", bufs=1) as consts,
            tc.tile_pool(name="work", bufs=2) as work,
            tc.tile_pool(name="sc", bufs=2) as sc,
            tc.tile_pool(name="ps_s", bufs=2, space="PSUM") as ps_s,
            tc.tile_pool(name="ps_ss", bufs=2, space="PSUM") as ps_ss,
            tc.tile_pool(name="ps_op", bufs=1, space="PSUM") as ps_op,
        ):
            # ---------------- residents ----------------
            w1_sb = consts.tile([C, KT, 2 * C], BF16)
            r1_sb = consts.tile([C, 2 * C], BF16)
            nc.sync.dma_start(out=w1_sb[:], in_=w1T[:].rearrange("t c o -> c t o"))
            nc.sync.dma_start(out=r1_sb[:], in_=res1[:])
            wlr_sb = consts.tile([C, UPC, K1, K1], BF16)
            wrt_sb = consts.tile([C, UPC, K1, K1 + C], BF16)
            nc.sync.dma_start(out=wlr_sb[:], in_=wlr[:].rearrange("j k c x -> c j k x"))
            nc.sync.dma_start(out=wrt_sb[:], in_=wrt[:].rearrange("j k c x -> c j k x"))
            if not triv_gb:
                gb_sb = consts.tile([C, PAIRS_PC, 2, N], BF16)
                nc.sync.dma_start(
                    out=gb_sb[:], in_=gbT[:].rearrange("q g c n -> c q g n"))
            id128 = consts.tile([128, 128], BF16)
            make_identity(nc, id128)
            ones64x1 = consts.tile([C, 1], F32)
            nc.gpsimd.memset(ones64x1, 1.0)
            ones1x64f = consts.tile([1, C], F32)
            nc.gpsimd.memset(ones1x64f, 1.0)
            eps_sb = consts.tile([1, 1], F32)
            nc.gpsimd.memset(eps_sb, 1e-6)

            wxo_t = [consts.tile([128, C + 1], BF16, name=f"wxo{jt}")
                     for jt in range(NT)]
            for jt in range(NT):
                nc.gpsimd.memset(wxo_t[jt][:, C : C + 1], 1.0)

            for _rep in range(reps):
                AL_DMA_ENG = {"sync": nc.sync, "vector": nc.vector,
                              "gpsimd": nc.gpsimd,
                              "scalar": nc.scalar}[
                    os.environ.get("K4_ALQ", "scalar")]
                NORM_ENG = (nc.gpsimd if os.environ.get("K4_NORM", "vector")
                            == "gpsimd" else nc.vector)
                mk = tc.alloc_tile_pool(name="mk", bufs=1)
                xs_sb = mk.tile([C, NSL, N], BF16)  # GLU outputs
                xw_ts = []
                for s in range(NSL):
                    xw_t = mk.tile([C, KT, N], BF16, tag="xwp", bufs=1,
                                   name=f"xw_t{s}")
                    nc.sync.dma_start(out=xw_t[:], in_=xw[s])
                    xw_ts.append(xw_t)
                mrel_sb = mk.tile([128, R, NT, N], BF16)
                msup_k = [mk.tile([128, NT, N], BF16, name=f"msup{k}")
                          for k in range(K1)]
                un_k = [mk.tile([128, NT, N], FP8, name=f"un{k}")
                        for k in range(K1)]
                mrelR = mrelT[:].rearrange("r (t p) n -> p r t n", p=128)
                msupR = [msupT[k].rearrange("(t p) n -> p t n", p=128)
                         for k in range(K1)]
                unR = [unegT[k].rearrange("(t p) n -> p t n", p=128)
                       for k in range(K1)]
                # k=0 masks in jt chunks first (earliest compute), then k=1,2
                for jt0 in range(0, NT, 2):
                    sl = slice(jt0, jt0 + 2)
                    for r in range(R):
                        nc.sync.dma_start(out=mrel_sb[:, r, sl, :],
                                          in_=mrelR[:, r, sl, :])
                    nc.sync.dma_start(out=msup_k[0][:, sl, :],
                                      in_=msupR[0][:, sl, :])
                    nc.sync.dma_start(out=un_k[0][:, sl, :],
                                      in_=unR[0][:, sl, :])
                for k in (1, 2):
                    nc.sync.dma_start(out=msup_k[k][:], in_=msupR[k])
                    nc.sync.dma_start(out=un_k[k][:], in_=unR[k])

                def glu_conv1(rhs, out_tile):
                    for f in range(NF):
                        cps = ps_ss.tile([2 * C, FCH], F32, tag="ss")
                        for tau in range(KT):
                            nc.tensor.matmul(
                                out=cps, lhsT=w1_sb[:, tau, :],
                                rhs=rhs(tau)[:, f * FCH : (f + 1) * FCH],
                                start=(tau == 0), stop=False,
                            )
                        nc.tensor.matmul(
                            out=cps, lhsT=r1_sb,
                            rhs=rhs(KT - 1)[:, f * FCH : (f + 1) * FCH],
                            start=False, stop=True,
                        )
                        sg = work.tile([C, FCH], BF16, tag="sg")
                        nc.scalar.activation(out=sg, in_=cps[C:, :], func=AF.Sigmoid)
                        pp = work.tile([C, FCH], BF16, tag="pp")
                        nc.scalar.copy(out=pp, in_=cps[:C, :])
                        nc.vector.tensor_mul(
                            out=out_tile[:, f * FCH : (f + 1) * FCH], in0=pp, in1=sg
                        )

                # ---------------- phase G: GLU1 for 3 slices ----------------
                for s in range(NSL):
                    glu_conv1(lambda tau, s=s: xw_ts[s][:, tau, :], xs_sb[:, s, :])

                # ---------------- phase A: attention units ------------------
                elus = []
                for j in range(UPC):
                    sdx = J2S[j]
                    xsT = xs_sb[:, sdx, :]
                    accT = work.tile([C, N], BF16, tag="accT", bufs=1)
                    for k in range(K1):
                        al_sb = work.tile([K1, N], BF16, tag="al_sb", bufs=1)
                        for f in range(NF):
                            alp = ps_s.tile([K1, FCH], F32, tag="alp", bufs=1)
                            nc.tensor.matmul(
                                out=alp, lhsT=wlr_sb[:, j, k, :],
                                rhs=xsT[:, f * FCH : (f + 1) * FCH],
                                start=True, stop=True,
                            )
                            nc.scalar.copy(
                                out=al_sb[:, f * FCH : (f + 1) * FCH], in_=alp
                            )
                        al1 = work.tile([1, N], BF16, tag="al1", bufs=1)
                        al2 = work.tile([1, N], BF16, tag="al2", bufs=1)
                        AL_DMA_ENG.dma_start(out=al1, in_=al_sb[1:2, :])
                        AL_DMA_ENG.dma_start(out=al2, in_=al_sb[2:3, :])
                        al_srcs = [al_sb[0:1, :], al1[:], al2[:]]
                        albc = [work.tile([128, N], BF16, tag=f"albc{r}", bufs=2,
                                          name=f"albc{r}") for r in range(K1)]
                        for r in range(K1):
                            nc.gpsimd.partition_broadcast(albc[r][:], al_srcs[r])
                        ops = [ps_op.tile([C + 1, FCH], F32, tag=f"op{f}",
                                          name=f"op{f}") for f in range(NF)]
                        for jt in range(NT):
                            awp = ps_s.tile([128, K1 + C], F32, tag="awp")
                            nc.tensor.matmul(
                                out=awp, lhsT=xsT[:, jt * 128 : (jt + 1) * 128],
                                rhs=wrt_sb[:, j, k, :], start=True, stop=True,
                            )
                            ar_sb = work.tile([128, K1], F32, tag="ar_sb", bufs=2)
                            nc.scalar.copy(out=ar_sb, in_=awp[:, :K1])
                            nc.scalar.copy(out=wxo_t[jt][:, :C], in_=awp[:, K1:])
                            ts = []
                            for r in range(K1):
                                vv = sc.tile([128, N], BF16, tag="vv",
                                             bufs=SC_BUFS)
                                nc.vector.tensor_scalar_add(
                                    vv, albc[r], ar_sb[:, r : r + 1]
                                )
                                tt = sc.tile([128, N], BF16, tag=f"t{r}",
                                             bufs=SC_BUFS)
                                msk = (mrel_sb[:, r, jt, :] if r < R
                                       else msup_k[k][:, jt, :])
                                eng = (nc.gpsimd if (r == 2 and
                                       jt % 2 == 1 and POOL_R2)
                                       else nc.vector)
                                eng.tensor_mul(out=tt, in0=vv, in1=msk)
                                ts.append(tt)
                            xe = sc.tile([128, N], BF16, tag="xe",
                                         bufs=SC_BUFS)
                            for f in range(NF):
                                ssp = ps_ss.tile([128, FCH], F32, tag="ss")
                                for r in range(K1):
                                    nc.tensor.matmul(
                                        out=ssp, lhsT=id128,
                                        rhs=ts[r][:, f * FCH : (f + 1) * FCH],
                                        start=(r == 0), stop=(r == K1 - 1),
                                    )
                                nc.scalar.activation(
                                    out=xe[:, f * FCH : (f + 1) * FCH], in_=ssp,
                                    func=AF.Exp,
                                )
                                nc.tensor.matmul(
                                    out=ops[f], lhsT=wxo_t[jt],
                                    rhs=xe[:, f * FCH : (f + 1) * FCH],
                                    start=(jt == 0), stop=False,
                                )
                                nc.tensor.matmul(
                                    out=ops[f], lhsT=wxo_t[jt],
                                    rhs=un_k[k][:, jt, f * FCH : (f + 1) * FCH],
                                    start=False, stop=(jt == NT - 1),
                                )
                        # normalize: accT += num * (1/den) broadcast
                        den_sb = work.tile([1, N], F32, tag="den_sb", bufs=1)
                        for f in range(NF):
                            nc.scalar.copy(
                                out=den_sb[:, f * FCH : (f + 1) * FCH],
                                in_=ops[f][C : C + 1, :],
                            )
                        rcp = work.tile([1, N], F32, tag="rcp", bufs=1)
                        nc.vector.reciprocal_approx_fast(out=rcp, in_=den_sb)
                        rcp16 = work.tile([1, N], BF16, tag="rcp16", bufs=1)
                        NORM_ENG.tensor_copy(out=rcp16, in_=rcp)
                        rcpb = work.tile([C, N], BF16, tag="rcpb", bufs=2)
                        nc.gpsimd.partition_broadcast(rcpb[:], rcp16[:])
                        num_sb = work.tile([C, N], BF16, tag="num")
                        for f in range(NF):
                            nc.scalar.copy(
                                out=num_sb[:, f * FCH : (f + 1) * FCH],
                                in_=ops[f][:C, :],
                            )
                        ne = NORM_ENG if k < K1 - 1 else nc.vector
                        if k == 0:
                            ne.tensor_mul(out=accT, in0=num_sb, in1=rcpb)
                        else:
                            tsc = work.tile([C, N], BF16, tag="tsc", bufs=1)
                            ne.tensor_mul(out=tsc, in0=num_sb, in1=rcpb)
                            ne.tensor_add(out=accT, in0=accT, in1=tsc)
                    # elu(accT) = relu(a) + exp(min(a,0)) - 1
                    mn = work.tile([C, N], BF16, tag="mn", bufs=1)
                    nc.vector.tensor_scalar_min(mn, accT, 0.0)
                    ex = work.tile([C, N], BF16, tag="ex", bufs=1)
                    nc.scalar.activation(out=ex, in_=mn, func=AF.Exp)
                    rl = work.tile([C, N], BF16, tag="rl", bufs=1)
                    nc.vector.tensor_scalar_max(rl, accT, 0.0)
                    er = work.tile([C, N], BF16, tag="mn", bufs=1)
                    nc.vector.tensor_add(out=er, in0=ex, in1=rl)
                    elu = work.tile([C, N], BF16, tag="elu", bufs=2)
                    nc.vector.tensor_scalar_add(elu, er, -1.0)
                    elus.append(elu)
                    if j in (2, 4):
                        q = j // 2 - 1
                        avq = work.tile([C, N], BF16, tag=f"av{q}", bufs=1,
                                        name=f"av{q}")
                        nc.vector.tensor_add(out=avq, in0=elus[j - 1],
                                             in1=elus[j])
                        nc.sync.dma_start(out=ag_in[q], in_=avq)
                        nc.gpsimd.collective_compute(
                            "AllGather", ALU.bypass,
                            replica_groups=[list(range(n_cores))],
                            ins=[ag_in[q : q + 1]], outs=[agos_av[q][:]],
                        )
                    elif j == 0:
                        nc.sync.dma_start(out=ag_in[2], in_=elu)
                        nc.gpsimd.collective_compute(
                            "AllGather", ALU.bypass,
                            replica_groups=[list(range(n_cores))],
                            ins=[ag_in[2:3]], outs=[agos_ex[:]],
                        )
                mk.release()

                # -------- tail: 2 pairs/core via zero-padded 20-tap GLU2 -----
                tl = tc.alloc_tile_pool(name="tl", bufs=1)
                w2s_sb = tl.tile([C, PAIRS_PC, NSLICE, 2 * C], BF16)
                nc.sync.dma_start(
                    out=w2s_sb[:], in_=w2sel[:].rearrange("q s c o -> c q s o"))
                av_sb = tl.tile([C, NSLICE, N], BF16)
                for s in range(16, NSLICE):
                    i = s - 16
                    a0 = tl.tile([C, N], BF16, tag="ga0", bufs=2)
                    nc.sync.dma_start(out=a0, in_=agos_ex[2 * i])
                    a1 = tl.tile([C, N], BF16, tag="ga1", bufs=2)
                    nc.sync.dma_start(out=a1, in_=agos_ex[2 * i + 1])
                    nc.vector.tensor_add(out=av_sb[:, s, :], in0=a0, in1=a1)
                for s in list(range(0, 16, 2)) + list(range(1, 16, 2)):
                    nc.sync.dma_start(out=av_sb[:, s, :],
                                      in_=agos_av[s % 2][s // 2])
                h2_sb = tl.tile([C, PAIRS_PC, N], BF16)
                for q in range(PAIRS_PC):
                    for f in range(NF):
                        cps = ps_ss.tile([2 * C, FCH], F32, tag="ss")
                        tap_order = (list(range(16, NSLICE))
                                     + list(range(0, 16, 2))
                                     + list(range(1, 16, 2)))
                        for si, s in enumerate(tap_order):
                            nc.tensor.matmul(
                                out=cps, lhsT=w2s_sb[:, q, s, :],
                                rhs=av_sb[:, s, f * FCH : (f + 1) * FCH],
                                start=(si == 0), stop=(si == NSLICE - 1),
                            )
                        sg = work.tile([C, FCH], BF16, tag="sg")
                        nc.scalar.activation(out=sg, in_=cps[C:, :], func=AF.Sigmoid)
                        pp = work.tile([C, FCH], BF16, tag="pp")
                        nc.vector.tensor_copy(out=pp, in_=cps[:C, :])
                        nc.vector.tensor_mul(
                            out=h2_sb[:, q, f * FCH : (f + 1) * FCH],
                            in0=pp, in1=sg,
                        )
                # stats + normalize for our 2 pairs
                stat_sb = tl.tile([1, 2 * PAIRS_PC], F32, tag="stats", bufs=1)
                for q in range(PAIRS_PC):
                    h2 = h2_sb[:, q, :]
                    sums = work.tile([C, 1], F32, tag="sums")
                    nc.vector.tensor_reduce(out=sums, in_=h2, axis=AX.X, op=ALU.add)
                    sq = tl.tile([C, N], BF16, tag="sqr", bufs=1)
                    nc.vector.tensor_mul(out=sq, in0=h2, in1=h2)
                    sqs = work.tile([C, 1], F32, tag="sqs")
                    nc.vector.tensor_reduce(out=sqs, in_=sq, axis=AX.X, op=ALU.add)
                    pair2 = work.tile([C, 2], F32, tag="pair2")
                    nc.scalar.copy(out=pair2[:, 0:1], in_=sums)
                    nc.scalar.copy(out=pair2[:, 1:2], in_=sqs)
                    totp = ps_s.tile([1, 2], F32, tag="alp", bufs=1, name="totp")
                    nc.tensor.matmul(out=totp, lhsT=ones64x1, rhs=pair2,
                                     start=True, stop=True)
                    nc.scalar.copy(out=stat_sb[:, 2 * q : 2 * q + 2], in_=totp)
                mu = work.tile([1, PAIRS_PC], F32, tag="mu", bufs=1)
                nc.scalar.activation(out=mu, in_=stat_sb[0:1, 0 : 2 * PAIRS_PC : 2],
                                     func=AF.Identity, scale=1.0 / NC_ELEMS)
                es = work.tile([1, PAIRS_PC], F32, tag="es", bufs=1)
                nc.scalar.activation(out=es, in_=stat_sb[0:1, 1 : 2 * PAIRS_PC : 2],
                                     func=AF.Identity, scale=1.0 / NC_ELEMS)
                musq = work.tile([1, PAIRS_PC], F32, tag="musq", bufs=1)
                nc.vector.tensor_mul(out=musq, in0=mu, in1=mu)
                varp = work.tile([1, PAIRS_PC], F32, tag="varp", bufs=1)
                nc.vector.tensor_sub(out=varp, in0=es, in1=musq)
                sd = work.tile([1, PAIRS_PC], F32, tag="sd", bufs=1)
                nc.scalar.activation(out=sd, in_=varp, func=AF.Sqrt, bias=eps_sb)
                rstd = work.tile([1, PAIRS_PC], F32, tag="rstd", bufs=1)
                nc.vector.reciprocal_approx_fast(out=rstd, in_=sd)
                nmr = work.tile([1, PAIRS_PC], F32, tag="nmr", bufs=1)
                nc.vector.tensor_mul(out=nmr, in0=mu, in1=rstd)
                nc.scalar.mul(nmr, nmr, -1.0)
                sb2 = work.tile([1, 2 * PAIRS_PC], F32, tag="sb2", bufs=1)
                nc.scalar.copy(out=sb2[:, 0 : 2 * PAIRS_PC : 2], in_=rstd)
                nc.scalar.copy(out=sb2[:, 1 : 2 * PAIRS_PC : 2], in_=nmr)
                bcp = ps_s.tile([C, 2 * PAIRS_PC], F32, tag="alp", bufs=1,
                                name="bcp")
                nc.tensor.matmul(out=bcp, lhsT=ones1x64f, rhs=sb2,
                                 start=True, stop=True)
                bc = work.tile([C, 2 * PAIRS_PC], F32, tag="bc", bufs=1)
                nc.scalar.copy(out=bc, in_=bcp)
                for q in range(PAIRS_PC):
                    og = tl.tile([C, N], F32, tag="og", bufs=1)
                    nc.scalar.activation(
                        out=og, in_=h2_sb[:, q, :], func=AF.Identity,
                        scale=bc[:, 2 * q : 2 * q + 1],
                        bias=bc[:, 2 * q + 1 : 2 * q + 2],
                    )
                    if not triv_gb:
                        nc.vector.tensor_mul(out=og, in0=og, in1=gb_sb[:, q, 0, :])
                        nc.vector.tensor_add(out=og, in0=og, in1=gb_sb[:, q, 1, :])
                    nc.sync.dma_start(out=out[q], in_=og)
                tl.release()
    if not nc.is_finalized():
        nc.finalize()
    return nc


def _prep(inputs, n_cores):
    bf16 = ml_dtypes.bfloat16
    fp8 = ml_dtypes.float8_e4m3fn
    x = np.asarray(inputs["x"], np.float32)
    supports = np.asarray(inputs["supports"], np.float32)
    atten = np.asarray(inputs["atten_supports"], np.float32)
    w_t1 = np.asarray(inputs["w_t1"], np.float32)
    Wt = np.asarray(inputs["Wt"], np.float32)
    Wl = np.asarray(inputs["Wl"], np.float32)
    Wr = np.asarray(inputs["Wr"], np.float32)
    w_t2 = np.asarray(inputs["w_t2"], np.float32)
    gamma = np.asarray(inputs["gamma"], np.float32)
    beta = np.asarray(inputs["beta"], np.float32)

    xT = np.ascontiguousarray(x.transpose(0, 1, 3, 2)).astype(bf16)  # [B,T,C,N]
    w1T = np.ascontiguousarray(w_t1[:, :, :, 0].transpose(2, 1, 0)).astype(bf16)
    w2Tf = np.ascontiguousarray(
        (0.5 * w_t2[:, :, :, 0]).transpose(2, 1, 0))          # [KT,C,2C] f32
    eye = np.eye(C, dtype=np.float32)
    res1 = np.concatenate([eye, 0 * eye], axis=1).astype(bf16)          # [C,2C]
    res05f = np.concatenate([0.5 * eye, 0 * eye], axis=1)
    mrel = (atten != 0).astype(np.float32)
    msup = (supports != 0).astype(np.float32)
    uni = np.minimum(mrel[0] + mrel[1] + msup, 1.0)                     # [K1,N,N]
    mrelT = np.ascontiguousarray(mrel.transpose(0, 2, 1)).astype(bf16)
    msupT = np.ascontiguousarray(msup.transpose(0, 2, 1)).astype(bf16)
    unegT = np.ascontiguousarray((uni - 1.0).transpose(0, 2, 1)).astype(fp8)

    in_maps = []
    for c in range(n_cores):
        units = [32 + c, 4 * c, 4 * c + 1, 4 * c + 2, 4 * c + 3]
        slices = sorted({u // 2 for u in units})
        assert len(slices) == NSL
        xwa = np.empty((NSL, C, KT, N), bf16)
        for i, s in enumerate(slices):
            b, t1 = s // T1, s % T1
            for tau in range(KT):
                xwa[i, :, tau] = xT[b, t1 + tau]
        wlrA = np.empty((UPC, K1, C, K1), np.float32)
        wrtA = np.empty((UPC, K1, C, K1 + C), np.float32)
        for j, u in enumerate(units):
            h = u % 2
            assert u // 2 == slices[J2S[j]]
            for k in range(K1):
                wlrA[j, k] = Wt[h, k] @ Wl[h, k].T
                wrtA[j, k, :, :K1] = Wt[h, k] @ Wr[h, k].T
                wrtA[j, k, :, K1:] = Wt[h, k]
        # zero-padded GLU2 taps for this core's pairs {2c, 2c+1}
        w2selA = np.zeros((PAIRS_PC, NSLICE, C, 2 * C), np.float32)
        gbA = np.empty((PAIRS_PC, 2, C, N), np.float32)
        for q in range(PAIRS_PC):
            p = PAIRS_PC * c + q
            b, t2 = p // T2, p % T2
            sbase = b * T1 + t2
            for tau in range(KT):
                w2selA[q, sbase + tau] += w2Tf[tau]
            w2selA[q, sbase + KT - 1] += res05f
            gbA[q, 0] = gamma[0, 0].T
            gbA[q, 1] = beta[0, 0].T
        in_maps.append(dict(
            xw=xwa, w1T=w1T, res1=res1,
            wlr=wlrA.astype(bf16), wrt=wrtA.astype(bf16),
            mrelT=mrelT, msupT=msupT, unegT=unegT,
            w2sel=w2selA.astype(bf16), gbT=gbA.astype(bf16),
        ))
    triv = bool(np.all(gamma == 1.0) and np.all(beta == 0.0))
    return in_maps, triv


def _assemble(outs):
    """outs: per-core [PAIRS_PC, C, N] arrays -> full [B, T2, C, N]."""
    o = np.empty((B, T2, C, N), np.float32)
    for c in range(len(outs)):
        for q in range(PAIRS_PC):
            p = PAIRS_PC * c + q
            o[p // T2, p % T2] = outs[c][q]
    return o


LAST = None


def kernel(**inputs):
    global LAST
    from concourse.bass_utils import run_bass_kernel_spmd

    in_maps, triv = _prep(inputs, N_CORES)
    key = (N_CORES, triv)
    if key not in _cache:
        _cache[key] = _build(N_CORES, triv_gb=triv)
    nc = _cache[key]
    res = run_bass_kernel_spmd(nc, in_maps, list(range(N_CORES)))
    LAST = res
    o = _assemble([np.asarray(res.results[c]["out"], np.float32)
                   for c in range(N_CORES)])
    return np.ascontiguousarray(o.transpose(0, 1, 3, 2))


# revision 4
# speedup vs baseline: 1.0496x; 1.0496x over previous
"""STConvBlock Trainium2 kernel v4.

vs v2: resident fp8 uneg masks (no per-(j,k) streaming), al staged with one
DMA + partition broadcasts, ELU relu on DVE, and a SHARDED TAIL: core c
computes only output pairs {2c, 2c+1} via a zero-padded 20-tap GLU2 conv
(per-core weights w2sel bake the 3 taps + residual at the right slice
offsets), writing a per-core [2, C, N] output that the host assembles.

Sharding: 40 (slice, head) units; core c owns units [4c..4c+3, 32+c].
AllGather per local unit j into shared agos[j].

Score math per (slice xs, head, cheb k), tile [128 j, 1024 i] (S transposed):
  v_r = al_r[i] + ar_r[j]    (DVE tensor_scalar: al broadcast + per-part ar)
  t_r = v_r * m_r            (DVE tensor_tensor, masks resident bf16)
  ss  = t0+t1+t2             (PE: 3 identity-matmul injects into f32 PSUM)
  X   = exp(ss)              (ACT, PSUM -> SBUF bf16; off-union X == 1.0)
  num/den += wxo^T @ X + wxo^T @ (u-1)   (PE; exact cancel off-union)
"""

import os
import numpy as np
import ml_dtypes

B, T, N, C = 2, 12, 1024, 64
KT = 3
T1 = T - KT + 1   # 10
T2 = T1 - KT + 1  # 8
H, K1, R = 2, 3, 2
NSLICE = B * T1       # 20
NUNITS = NSLICE * H   # 40
N_CORES = 8
NT = N // 128         # 8
FCH = 512
NF = N // FCH         # 2
NC_ELEMS = float(N * C)
UPC = NUNITS // N_CORES  # 5
NSL = 3                  # distinct slices per core
PAIRS_PC = 2             # tail (b,t2) pairs per core
J2S = [2, 0, 0, 1, 1]

_cache = {}
POOL_R2 = int(os.environ.get('K4_POOL_R2', '1'))


def _build(n_cores, triv_gb=True, debug=False, reps=1):
    import concourse.bass as bass
    import concourse.tile as tile
    import concourse.mybir as mybir
    from concourse import bacc
    from concourse.masks import make_identity

    F32 = mybir.dt.float32
    BF16 = mybir.dt.bfloat16
    FP8 = mybir.dt.float8e4
    AF = mybir.ActivationFunctionType
    ALU = mybir.AluOpType
    AX = mybir.AxisListType

    nc = bacc.Bacc(None, target_bir_lowering=False)
    xw = nc.dram_tensor("xw", [NSL, C, KT, N], BF16, kind="ExternalInput")
    w1T = nc.dram_tensor("w1T", [KT, C, 2 * C], BF16, kind="ExternalInput")
    res1 = nc.dram_tensor("res1", [C, 2 * C], BF16, kind="ExternalInput")
    wlr = nc.dram_tensor("wlr", [UPC, K1, C, K1], BF16, kind="ExternalInput")
    wrt = nc.dram_tensor("wrt", [UPC, K1, C, K1 + C], BF16, kind="ExternalInput")
    mrelT = nc.dram_tensor("mrelT", [R, N, N], BF16, kind="ExternalInput")
    msupT = nc.dram_tensor("msupT", [K1, N, N], BF16, kind="ExternalInput")
    unegT = nc.dram_tensor("unegT", [K1, N, N], FP8, kind="ExternalInput")
    # per-core zero-padded GLU2 weights: taps + residual at slice offsets
    w2sel = nc.dram_tensor("w2sel", [PAIRS_PC, NSLICE, C, 2 * C], BF16,
                           kind="ExternalInput")
    gbT = nc.dram_tensor("gbT", [PAIRS_PC, 2, C, N], BF16, kind="ExternalInput")
    out = nc.dram_tensor("out", [PAIRS_PC, C, N], F32, kind="ExternalOutput")
    ag_in = nc.dram_tensor("ag_in", [3, C, N], BF16)
    # agos_av[q][c] = av slice 2c+q (head-summed, pre-averaged by w2sel);
    # agos_ex[c] = unit 32+c (slices 16..19, heads split across cores)
    agos_av = [nc.dram_tensor(f"agoav{q}", [N_CORES, C, N], BF16,
                              addr_space="Shared") for q in range(2)]
    agos_ex = nc.dram_tensor("agoex", [N_CORES, C, N], BF16,
                             addr_space="Shared")

    with tile.TileContext(nc) as tc:
        with (
            tc.tile_pool(name="consts", bufs=1) as consts,
            tc.tile_pool(name="work", bufs=2) as work,
            tc.tile_pool(name="sc", bufs=2) as sc,
            tc.tile_pool(name="ps_s", bufs=2, space="PSUM") as ps_s,
            tc.tile_pool(name="ps_ss", bufs=2, space="PSUM") as ps_ss,
            tc.tile_pool(name="ps_op", bufs=1, space="PSUM") as ps_op,
        ):
            # ---------------- residents ----------------
            w1_sb = consts.tile([C, KT, 2 * C], BF16)
            r1_sb = consts.tile([C, 2 * C], BF16)
            nc.sync.dma_start(out=w1_sb[:], in_=w1T[:].rearrange("t c o -> c t o"))
            nc.sync.dma_start(out=r1_sb[:], in_=res1[:])
            wlr_sb = consts.tile([C, UPC, K1, K1], BF16)
            wrt_sb = consts.tile([C, UPC, K1, K1 + C], BF16)
            nc.sync.dma_start(out=wlr_sb[:], in_=wlr[:].rearrange("j k c x -> c j k x"))
            nc.sync.dma_start(out=wrt_sb[:], in_=wrt[:].rearrange("j k c x -> c j k x"))
            if not triv_gb:
                gb_sb = consts.tile([C, PAIRS_PC, 2, N], BF16)
                nc.sync.dma_start(
                    out=gb_sb[:], in_=gbT[:].rearrange("q g c n -> c q g n"))
            id128 = consts.tile([128, 128], BF16)
            make_identity(nc, id128)
            ones64x1 = consts.tile([C, 1], F32)
            nc.gpsimd.memset(ones64x1, 1.0)
            ones1x64f = consts.tile([1, C], F32)
            nc.gpsimd.memset(ones1x64f, 1.0)
            eps_sb = consts.tile([1, 1], F32)
            nc.gpsimd.memset(eps_sb, 1e-6)

            wxo_t = [consts.tile([128, C + 1], BF16, name=f"wxo{jt}")
                     for jt in range(NT)]
            for jt in range(NT):
                nc.gpsimd.memset(wxo_t[jt][:, C : C + 1], 1.0)

            for _rep in range(reps):
                AL_DMA_ENG = {"sync": nc.sync, "vector": nc.vector,
                              "gpsimd": nc.gpsimd,
                              "scalar": nc.scalar}[
                    os.environ.get("K4_ALQ", "scalar")]
                NORM_ENG = (nc.gpsimd if os.environ.get("K4_NORM", "vector")
                            == "gpsimd" else nc.vector)
                mk = tc.alloc_tile_pool(name="mk", bufs=1)
                xs_sb = mk.tile([C, NSL, N], BF16)  # GLU outputs
                xw_ts = []
                for s in range(NSL):
                    xw_t = mk.tile([C, KT, N], BF16, tag="xwp", bufs=2,
                                   name=f"xw_t{s}")
                    nc.sync.dma_start(out=xw_t[:], in_=xw[s])
                    xw_ts.append(xw_t)
                mrel_sb = mk.tile([128, R, NT, N], BF16)
                msup_k = [mk.tile([128, NT, N], BF16, name=f"msup{k}")
                          for k in range(K1)]
                un_k = [mk.tile([128, NT, N], FP8, name=f"un{k}")
                        for k in range(K1)]
                mrelR = mrelT[:].rearrange("r (t p) n -> p r t n", p=128)
                msupR = [msupT[k].rearrange("(t p) n -> p t n", p=128)
                         for k in range(K1)]
                unR = [unegT[k].rearrange("(t p) n -> p t n", p=128)
                       for k in range(K1)]
                # k=0 masks in jt chunks first (earliest compute), then k=1,2
                for jt0 in range(0, NT, 2):
                    sl = slice(jt0, jt0 + 2)
                    for r in range(R):
                        nc.sync.dma_start(out=mrel_sb[:, r, sl, :],
                                          in_=mrelR[:, r, sl, :])
                    nc.sync.dma_start(out=msup_k[0][:, sl, :],
                                      in_=msupR[0][:, sl, :])
                    nc.sync.dma_start(out=un_k[0][:, sl, :],
                                      in_=unR[0][:, sl, :])
                for k in (1, 2):
                    nc.sync.dma_start(out=msup_k[k][:], in_=msupR[k])
                    nc.sync.dma_start(out=un_k[k][:], in_=unR[k])

                def glu_conv1(rhs, out_tile):
                    for f in range(NF):
                        cps = ps_ss.tile([2 * C, FCH], F32, tag="ss")
                        for tau in range(KT):
                            nc.tensor.matmul(
                                out=cps, lhsT=w1_sb[:, tau, :],
                                rhs=rhs(tau)[:, f * FCH : (f + 1) * FCH],
                                start=(tau == 0), stop=False,
                            )
                        nc.tensor.matmul(
                            out=cps, lhsT=r1_sb,
                            rhs=rhs(KT - 1)[:, f * FCH : (f + 1) * FCH],
                            start=False, stop=True,
                        )
                        sg = work.tile([C, FCH], BF16, tag="sg")
                        nc.scalar.activation(out=sg, in_=cps[C:, :], func=AF.Sigmoid)
                        pp = work.tile([C, FCH], BF16, tag="pp")
                        nc.scalar.copy(out=pp, in_=cps[:C, :])
                        nc.vector.tensor_mul(
                            out=out_tile[:, f * FCH : (f + 1) * FCH], in0=pp, in1=sg
                        )

                # ---------------- phase G: GLU1 for 3 slices ----------------
                for s in range(NSL):
                    glu_conv1(lambda tau, s=s: xw_ts[s][:, tau, :], xs_sb[:, s, :])

                # ---------------- phase A: attention units ------------------
                elus = []
                for j in range(UPC):
                    sdx = J2S[j]
                    xsT = xs_sb[:, sdx, :]
                    accT = work.tile([C, N], BF16, tag="accT", bufs=1)
                    for k in range(K1):
                        al_sb = work.tile([K1, N], BF16, tag="al_sb", bufs=1)
                        for f in range(NF):
                            alp = ps_s.tile([K1, FCH], F32, tag="alp", bufs=1)
                            nc.tensor.matmul(
                                out=alp, lhsT=wlr_sb[:, j, k, :],
                                rhs=xsT[:, f * FCH : (f + 1) * FCH],
                                start=True, stop=True,
                            )
                            nc.scalar.copy(
                                out=al_sb[:, f * FCH : (f + 1) * FCH], in_=alp
                            )
                        al1 = work.tile([1, N], BF16, tag="al1", bufs=1)
                        al2 = work.tile([1, N], BF16, tag="al2", bufs=1)
                        AL_DMA_ENG.dma_start(out=al1, in_=al_sb[1:2, :])
                        AL_DMA_ENG.dma_start(out=al2, in_=al_sb[2:3, :])
                        al_srcs = [al_sb[0:1, :], al1[:], al2[:]]
                        albc = [work.tile([128, N], BF16, tag=f"albc{r}", bufs=1,
                                          name=f"albc{r}") for r in range(K1)]
                        for r in range(K1):
                            nc.gpsimd.partition_broadcast(albc[r][:], al_srcs[r])
                        ops = [ps_op.tile([C + 1, FCH], F32, tag=f"op{f}",
                                          name=f"op{f}") for f in range(NF)]
                        for jt in range(NT):
                            awp = ps_s.tile([128, K1 + C], F32, tag="awp")
                            nc.tensor.matmul(
                                out=awp, lhsT=xsT[:, jt * 128 : (jt + 1) * 128],
                                rhs=wrt_sb[:, j, k, :], start=True, stop=True,
                            )
                            ar_sb = work.tile([128, K1], F32, tag="ar_sb", bufs=2)
                            nc.scalar.copy(out=ar_sb, in_=awp[:, :K1])
                            nc.scalar.copy(out=wxo_t[jt][:, :C], in_=awp[:, K1:])
                            ts = []
                            for r in range(K1):
                                vv = sc.tile([128, N], BF16, tag="vv",
                                             bufs=SC_BUFS)
                                nc.vector.tensor_scalar_add(
                                    vv, albc[r], ar_sb[:, r : r + 1]
                                )
                                tt = sc.tile([128, N], BF16, tag=f"t{r}",
                                             bufs=SC_BUFS)
                                msk = (mrel_sb[:, r, jt, :] if r < R
                                       else msup_k[k][:, jt, :])
                                eng = (nc.gpsimd if (r == 2 and
                                       jt % 2 == 1 and POOL_R2)
                                       else nc.vector)
                                eng.tensor_mul(out=tt, in0=vv, in1=msk)
                                ts.append(tt)
                            xe = sc.tile([128, N], BF16, tag="xe",
                                         bufs=SC_BUFS)
                            for f in range(NF):
                                ssp = ps_ss.tile([128, FCH], F32, tag="ss")
                                for r in range(K1):
                                    nc.tensor.matmul(
                                        out=ssp, lhsT=id128,
                                        rhs=ts[r][:, f * FCH : (f + 1) * FCH],
                                        start=(r == 0), stop=(r == K1 - 1),
                                    )
                                nc.scalar.activation(
                                    out=xe[:, f * FCH : (f + 1) * FCH], in_=ssp,
                                    func=AF.Exp,
                                )
                                nc.tensor.matmul(
                                    out=ops[f], lhsT=wxo_t[jt],
                                    rhs=xe[:, f * FCH : (f + 1) * FCH],
                                    start=(jt == 0), stop=False,
                                )
                                nc.tensor.matmul(
                                    out=ops[f], lhsT=wxo_t[jt],
                                    rhs=un_k[k][:, jt, f * FCH : (f + 1) * FCH],
                                    start=False, stop=(jt == NT - 1),
                                )
                        # normalize: accT += num * (1/den) broadcast
                        den_sb = work.tile([1, N], F32, tag="den_sb", bufs=1)
                        for f in range(NF):
                            nc.scalar.copy(
                                out=den_sb[:, f * FCH : (f + 1) * FCH],
                                in_=ops[f][C : C + 1, :],
                            )
                        rcp = work.tile([1, N], F32, tag="rcp", bufs=1)
                        nc.vector.reciprocal_approx_fast(out=rcp, in_=den_sb)
                        rcp16 = work.tile([1, N], BF16, tag="rcp16", bufs=1)
                        NORM_ENG.tensor_copy(out=rcp16, in_=rcp)
                        rcpb = work.tile([C, N], BF16, tag="rcpb", bufs=2)
                        nc.gpsimd.partition_broadcast(rcpb[:], rcp16[:])
                        num_sb = work.tile([C, N], BF16, tag="num")
                        for f in range(NF):
                            nc.scalar.copy(
                                out=num_sb[:, f * FCH : (f + 1) * FCH],
                                in_=ops[f][:C, :],
                            )
                        ne = NORM_ENG if k < K1 - 1 else nc.vector
                        if k == 0:
                            ne.tensor_mul(out=accT, in0=num_sb, in1=rcpb)
                        else:
                            tsc = work.tile([C, N], BF16, tag="tsc", bufs=1)
                            ne.tensor_mul(out=tsc, in0=num_sb, in1=rcpb)
                            ne.tensor_add(out=accT, in0=accT, in1=tsc)
                    # elu(accT) = relu(a) + exp(min(a,0)) - 1
                    mn = work.tile([C, N], BF16, tag="mn", bufs=1)
                    nc.vector.tensor_scalar_min(mn, accT, 0.0)
                    ex = work.tile([C, N], BF16, tag="ex", bufs=1)
                    nc.scalar.activation(out=ex, in_=mn, func=AF.Exp)
                    rl = work.tile([C, N], BF16, tag="rl", bufs=1)
                    nc.vector.tensor_scalar_max(rl, accT, 0.0)
                    er = work.tile([C, N], BF16, tag="mn", bufs=1)
                    nc.vector.tensor_add(out=er, in0=ex, in1=rl)
                    elu = work.tile([C, N], BF16, tag="elu", bufs=2)
                    nc.vector.tensor_scalar_add(elu, er, -1.0)
                    elus.append(elu)
                    if j in (2, 4):
                        q = j // 2 - 1
                        avq = work.tile([C, N], BF16, tag=f"av{q}", bufs=1,
                                        name=f"av{q}")
                        nc.vector.tensor_add(out=avq, in0=elus[j - 1],
                                             in1=elus[j])
                        nc.sync.dma_start(out=ag_in[q], in_=avq)
                        nc.gpsimd.collective_compute(
                            "AllGather", ALU.bypass,
                            replica_groups=[list(range(n_cores))],
                            ins=[ag_in[q : q + 1]], outs=[agos_av[q][:]],
                        )
                    elif j == 0:
                        nc.sync.dma_start(out=ag_in[2], in_=elu)
                        nc.gpsimd.collective_compute(
                            "AllGather", ALU.bypass,
                            replica_groups=[list(range(n_cores))],
                            ins=[ag_in[2:3]], outs=[agos_ex[:]],
                        )
                mk.release()

                # -------- tail: 2 pairs/core via zero-padded 20-tap GLU2 -----
                tl = tc.alloc_tile_pool(name="tl", bufs=1)
                w2s_sb = tl.tile([C, PAIRS_PC, NSLICE, 2 * C], BF16)
                nc.sync.dma_start(
                    out=w2s_sb[:], in_=w2sel[:].rearrange("q s c o -> c q s o"))
                av_sb = tl.tile([C, NSLICE, N], BF16)
                for s in range(16, NSLICE):
                    i = s - 16
                    a0 = tl.tile([C, N], BF16, tag="ga0", bufs=2)
                    nc.sync.dma_start(out=a0, in_=agos_ex[2 * i])
                    a1 = tl.tile([C, N], BF16, tag="ga1", bufs=2)
                    nc.sync.dma_start(out=a1, in_=agos_ex[2 * i + 1])
                    nc.vector.tensor_add(out=av_sb[:, s, :], in0=a0, in1=a1)
                for s in list(range(0, 16, 2)) + list(range(1, 16, 2)):
                    nc.sync.dma_start(out=av_sb[:, s, :],
                                      in_=agos_av[s % 2][s // 2])
                h2_sb = tl.tile([C, PAIRS_PC, N], BF16)
                for q in range(PAIRS_PC):
                    for f in range(NF):
                        cps = ps_ss.tile([2 * C, FCH], F32, tag="ss")
                        tap_order = (list(range(16, NSLICE))
                                     + list(range(0, 16, 2))
                                     + list(range(1, 16, 2)))
                        for si, s in enumerate(tap_order):
                            nc.tensor.matmul(
                                out=cps, lhsT=w2s_sb[:, q, s, :],
                                rhs=av_sb[:, s, f * FCH : (f + 1) * FCH],
                                start=(si == 0), stop=(si == NSLICE - 1),
                            )
                        sg = work.tile([C, FCH], BF16, tag="sg")
                        nc.scalar.activation(out=sg, in_=cps[C:, :], func=AF.Sigmoid)
                        pp = work.tile([C, FCH], BF16, tag="pp")
                        nc.vector.tensor_copy(out=pp, in_=cps[:C, :])
                        nc.vector.tensor_mul(
                            out=h2_sb[:, q, f * FCH : (f + 1) * FCH],
                            in0=pp, in1=sg,
                        )
                # stats + normalize for our 2 pairs
                stat_sb = tl.tile([1, 2 * PAIRS_PC], F32, tag="stats", bufs=1)
                for q in range(PAIRS_PC):
                    h2 = h2_sb[:, q, :]
                    sums = work.tile([C, 1], F32, tag="sums")
                    nc.vector.tensor_reduce(out=sums, in_=h2, axis=AX.X, op=ALU.add)
                    sq = tl.tile([C, N], BF16, tag="sqr", bufs=1)
                    nc.vector.tensor_mul(out=sq, in0=h2, in1=h2)
                    sqs = work.tile([C, 1], F32, tag="sqs")
                    nc.vector.tensor_reduce(out=sqs, in_=sq, axis=AX.X, op=ALU.add)
                    pair2 = work.tile([C, 2], F32, tag="pair2")
                    nc.scalar.copy(out=pair2[:, 0:1], in_=sums)
                    nc.scalar.copy(out=pair2[:, 1:2], in_=sqs)
                    totp = ps_s.tile([1, 2], F32, tag="alp", bufs=1, name="totp")
                    nc.tensor.matmul(out=totp, lhsT=ones64x1, rhs=pair2,
                                     start=True, stop=True)
                    nc.scalar.copy(out=stat_sb[:, 2 * q : 2 * q + 2], in_=totp)
                mu = work.tile([1, PAIRS_PC], F32, tag="mu", bufs=1)
                nc.scalar.activation(out=mu, in_=stat_sb[0:1, 0 : 2 * PAIRS_PC : 2],
                                     func=AF.Identity, scale=1.0 / NC_ELEMS)
                es = work.tile([1, PAIRS_PC], F32, tag="es", bufs=1)
                nc.scalar.activation(out=es, in_=stat_sb[0:1, 1 : 2 * PAIRS_PC : 2],
                                     func=AF.Identity, scale=1.0 / NC_ELEMS)
                musq = work.tile([1, PAIRS_PC], F32, tag="musq", bufs=1)
                nc.vector.tensor_mul(out=musq, in0=mu, in1=mu)
                varp = work.tile([1, PAIRS_PC], F32, tag="varp", bufs=1)
                nc.vector.tensor_sub(out=varp, in0=es, in1=musq)
                sd = work.tile([1, PAIRS_PC], F32, tag="sd", bufs=1)
                nc.scalar.activation(out=sd, in_=varp, func=AF.Sqrt, bias=eps_sb)
                rstd = work.tile([1, PAIRS_PC], F32, tag="rstd", bufs=1)
                nc.vector.reciprocal_approx_fast(out=rstd, in_=sd)
                nmr = work.tile([1, PAIRS_PC], F32, tag="nmr", bufs=1)
                nc.vector.tensor_mul(out=nmr, in0=mu, in1=rstd)
                nc.scalar.mul(nmr, nmr, -1.0)
                sb2 = work.tile([1, 2 * PAIRS_PC], F32, tag="sb2", bufs=1)
                nc.scalar.copy(out=sb2[:, 0 : 2 * PAIRS_PC : 2], in_=rstd)
                nc.scalar.copy(out=sb2[:, 1 : 2 * PAIRS_PC : 2], in_=nmr)
                bcp = ps_s.tile([C, 2 * PAIRS_PC], F32, tag="alp", bufs=1,
                                name="bcp")
                nc.tensor.matmul(out=bcp, lhsT=ones1x64f, rhs=sb2,
                                 start=True, stop=True)
                bc = work.tile([C, 2 * PAIRS_PC], F32, tag="bc", bufs=1)
                nc.scalar.copy(out=bc, in_=bcp)
                for q in range(PAIRS_PC):
                    og = tl.tile([C, N], F32, tag="og", bufs=1)
                    nc.scalar.activation(
                        out=og, in_=h2_sb[:, q, :], func=AF.Identity,
                        scale=bc[:, 2 * q : 2 * q + 1],
                        bias=bc[:, 2 * q + 1 : 2 * q + 2],
                    )
                    if not triv_gb:
                        nc.vector.tensor_mul(out=og, in0=og, in1=gb_sb[:, q, 0, :])
                        nc.vector.tensor_add(out=og, in0=og, in1=gb_sb[:, q, 1, :])
                    nc.sync.dma_start(out=out[q], in_=og)
                tl.release()
    if not nc.is_finalized():
        nc.finalize()
    return nc


def _prep(inputs, n_cores):
    bf16 = ml_dtypes.bfloat16
    fp8 = ml_dtypes.float8_e4m3fn
    x = np.asarray(inputs["x"], np.float32)
    supports = np.asarray(inputs["supports"], np.float32)
    atten = np.asarray(inputs["atten_supports"], np.float32)
    w_t1 = np.asarray(inputs["w_t1"], np.float32)
    Wt = np.asarray(inputs["Wt"], np.float32)
    Wl = np.asarray(inputs["Wl"], np.float32)
    Wr = np.asarray(inputs["Wr"], np.float32)
    w_t2 = np.asarray(inputs["w_t2"], np.float32)
    gamma = np.asarray(inputs["gamma"], np.float32)
    beta = np.asarray(inputs["beta"], np.float32)

    xT = np.ascontiguousarray(x.transpose(0, 1, 3, 2)).astype(bf16)  # [B,T,C,N]
    w1T = np.ascontiguousarray(w_t1[:, :, :, 0].transpose(2, 1, 0)).astype(bf16)
    w2Tf = np.ascontiguousarray(
        (0.5 * w_t2[:, :, :, 0]).transpose(2, 1, 0))          # [KT,C,2C] f32
    eye = np.eye(C, dtype=np.float32)
    res1 = np.concatenate([eye, 0 * eye], axis=1).astype(bf16)          # [C,2C]
    res05f = np.concatenate([0.5 * eye, 0 * eye], axis=1)
    mrel = (atten != 0).astype(np.float32)
    msup = (supports != 0).astype(np.float32)
    uni = np.minimum(mrel[0] + mrel[1] + msup, 1.0)                     # [K1,N,N]
    mrelT = np.ascontiguousarray(mrel.transpose(0, 2, 1)).astype(bf16)
    msupT = np.ascontiguousarray(msup.transpose(0, 2, 1)).astype(bf16)
    unegT = np.ascontiguousarray((uni - 1.0).transpose(0, 2, 1)).astype(fp8)

    in_maps = []
    for c in range(n_cores):
        units = [32 + c, 4 * c, 4 * c + 1, 4 * c + 2, 4 * c + 3]
        slices = sorted({u // 2 for u in units})
        assert len(slices) == NSL
        xwa = np.empty((NSL, C, KT, N), bf16)
        for i, s in enumerate(slices):
            b, t1 = s // T1, s % T1
            for tau in range(KT):
                xwa[i, :, tau] = xT[b, t1 + tau]
        wlrA = np.empty((UPC, K1, C, K1), np.float32)
        wrtA = np.empty((UPC, K1, C, K1 + C), np.float32)
        for j, u in enumerate(units):
            h = u % 2
            assert u // 2 == slices[J2S[j]]
            for k in range(K1):
                wlrA[j, k] = Wt[h, k] @ Wl[h, k].T
                wrtA[j, k, :, :K1] = Wt[h, k] @ Wr[h, k].T
                wrtA[j, k, :, K1:] = Wt[h, k]
        # zero-padded GLU2 taps for this core's pairs {2c, 2c+1}
        w2selA = np.zeros((PAIRS_PC, NSLICE, C, 2 * C), np.float32)
        gbA = np.empty((PAIRS_PC, 2, C, N), np.float32)
        for q in range(PAIRS_PC):
            p = PAIRS_PC * c + q
            b, t2 = p // T2, p % T2
            sbase = b * T1 + t2
            for tau in range(KT):
                w2selA[q, sbase + tau] += w2Tf[tau]
            w2selA[q, sbase + KT - 1] += res05f
            gbA[q, 0] = gamma[0, 0].T
            gbA[q, 1] = beta[0, 0].T
        in_maps.append(dict(
            xw=xwa, w1T=w1T, res1=res1,
            wlr=wlrA.astype(bf16), wrt=wrtA.astype(bf16),
            mrelT=mrelT, msupT=msupT, unegT=unegT,
            w2sel=w2selA.astype(bf16), gbT=gbA.astype(bf16),
        ))
    triv = bool(np.all(gamma == 1.0) and np.all(beta == 0.0))
    return in_maps, triv


def _assemble(outs):
    """outs: per-core [PAIRS_PC, C, N] arrays -> full [B, T2, C, N]."""
    o = np.empty((B, T2, C, N), np.float32)
    for c in range(len(outs)):
        for q in range(PAIRS_PC):
            p = PAIRS_PC * c + q
            o[p // T2, p % T2] = outs[c][q]
    return o


LAST = None


def kernel(**inputs):
    global LAST
    from concourse.bass_utils import run_bass_kernel_spmd

    in_maps, triv = _prep(inputs, N_CORES)
    key = (N_CORES, triv)
    if key not in _cache:
        _cache[key] = _build(N_CORES, triv_gb=triv)
    nc = _cache[key]
    res = run_bass_kernel_spmd(nc, in_maps, list(range(N_CORES)))
    LAST = res
    o = _assemble([np.asarray(res.results[c]["out"], np.float32)
                   for c in range(N_CORES)])
    return np.ascontiguousarray(o.transpose(0, 1, 3, 2))
